# revision 2
# baseline (speedup 1.0000x reference)
"""W4A16 group-quantized GEMM on 8 Trainium2 NeuronCores.

Problem: out[b,s,n] = x[b,s,:] @ dequant(W).T where W is INT4
group-quantized (group 128 along K), x is (4,4096,4096) fp16,
W is (11008, 4096) int4 + (11008, 32) fp16 scales.

Strategy (Megatron column-parallel): shard N=11008 -> 1376 per core,
replicate x. The GEMM runs in fp8 (e4m3) with DoubleRow perf mode
(2 k-tiles contracted per instruction at 0.5 cycles/row = 4x the
fp16 PE rate), using a two-pass error-compensated decomposition:

  x = x_hi + x_lo          (x_hi = fp8(x), x_lo = residual)
  w = w_hi + w_lo          (w_hi = fp8(w), w_lo = residual)
  c = fp8(x_lo + beta*x_hi),  d = fp8(w_hi + w_lo/beta),  beta = 1/8
  out = (1-beta)*(x_hi @ w_hi) + c @ d
      = x@w + (1/beta-1)*x_lo@w_lo + O(second-order)   [~0.7% rel err]

Both passes are fp8 DoubleRow matmuls, so the PE does 2*K/256 = 32
instructions per 128x512 output tile instead of fp16's 32 full-rate
ones: ~2x net speedup over the fp16 baseline. The two PSUM partials
are combined on the DVE as out = 0.875*P1 + P2 (the one PSUM->SBUF
copy the fp16 version needed anyway, plus one add).

Host-side prep computes the fp8 operand tensors (dequant + split) in
numpy and ships them transposed to (K, M)/(K, NC) so K lands on
partitions. Weights per core: 2 x 5.5MB fp8 resident in SBUF.
"""

import sys

import numpy as np

if "/opt/trn_rl_repo" not in sys.path:
    sys.path.insert(0, "/opt/trn_rl_repo")

import concourse.bass as bass
import concourse.mybir as mybir
import concourse.tile as tile

def _split_multiwaits_json(bir_json: bytes) -> bytes:
    """Walrus in this environment encodes at most ONE sync-wait per
    instruction; Tile emits several. Split extras onto preceding same-engine
    NoOps (engine executes in order, so blocking semantics are identical)."""
    import orjson

    m = orjson.loads(bir_json)
    for fn in m.get("functions", []):
        for blk in fn.get("blocks", []):
            insts = blk.get("instructions")
            if not insts:
                continue
            out = []
            for ins in insts:
                si = ins.get("sync_info")
                if si:
                    ow = si.get("on_wait") or []
                    if len(ow) > 1:
                        for i, w in enumerate(ow[:-1]):
                            out.append(
                                {
                                    "debug": ins.get("debug", 0),
                                    "engine": ins["engine"],
                                    "ins": [],
                                    "outs": [],
                                    "name": f"{ins['name']}-sw{i}",
                                    "opcode": "NoOp",
                                    "sync_info": {"on_update": [], "on_wait": [w]},
                                }
                            )
                        si["on_wait"] = [ow[-1]]
                out.append(ins)
            blk["instructions"] = out
    return orjson.dumps(m)


def _install_walrus_compat_patch():
    from concourse import bass2jax as b2j
    from concourse import bass_utils as bu

    if getattr(bu.compile_bir_kernel, "_mw_patched", False):
        return
    orig = bu.compile_bir_kernel

    def patched(bir_json, tmpdir, neff_name="file.neff"):
        return orig(_split_multiwaits_json(bir_json), tmpdir, neff_name=neff_name)

    patched._mw_patched = True
    bu.compile_bir_kernel = patched
    b2j.compile_bir_kernel = patched


_install_walrus_compat_patch()

P = 128
K = 4096
N = 11008
M = 16384  # 4 * 4096 tokens
GROUP = 128
KG = K // GROUP  # 32 scale groups
NCORES = 8
NC = N // NCORES  # 1376 output cols per core
KT = K // P  # 32 k-tiles
KP = KT // 2  # 16 DoubleRow k-pairs
MB = 512  # m rows per x DMA block
CHUNKS = [(0, 512), (512, 1024), (1024, 1376)]  # psum n-chunks
BETA = 0.125
LAM = 1.0 - BETA  # 0.875

F8 = mybir.dt.float8e4


def build_program(m_total: int = M, reps: int = 1, loop_reps: int = 1) -> bass.Bass:
    """reps>1 duplicates the main GEMM loop (output overwritten each rep);
    loop_reps>1 wraps it in a hardware For_i loop. Both are used only for
    differential timing of one epoch on hardware."""
    nc = bass.Bass()
    xqT = nc.declare_dram_parameter("xqT", [K, m_total], F8, isOutput=False)
    xcT = nc.declare_dram_parameter("xcT", [K, m_total], F8, isOutput=False)
    whT = nc.declare_dram_parameter("whT", [K, NC], F8, isOutput=False)
    wdT = nc.declare_dram_parameter("wdT", [K, NC], F8, isOutput=False)
    out = nc.declare_dram_parameter(
        "out", [m_total, NC], mybir.dt.float16, isOutput=True
    )

    with tile.TileContext(nc) as tc:
        with (
            tc.tile_pool(name="wres", bufs=1) as wpool,
            tc.tile_pool(name="xin", bufs=2) as xpool,
            tc.tile_pool(name="outsb", bufs=3) as opool,
            tc.tile_pool(name="psA", bufs=2, space="PSUM") as psA,
            tc.tile_pool(name="psB", bufs=2, space="PSUM") as psB,
        ):
            # ---- resident fp8 weight shards [P, KT, NC] ----
            wh = wpool.tile([P, KT, NC], F8)
            wd = wpool.tile([P, KT, NC], F8)
            whv = whT.rearrange("(ko p) n -> p ko n", p=P)
            wdv = wdT.rearrange("(ko p) n -> p ko n", p=P)
            for t in range(KT):
                nc.sync.dma_start(wh[:, t, :], whv[:, t, :])
                nc.sync.dma_start(wd[:, t, :], wdv[:, t, :])

            xqv = xqT.rearrange("(ko p) m -> p ko m", p=P)
            xcv = xcT.rearrange("(ko p) m -> p ko m", p=P)
            n_blocks = m_total // MB

            import contextlib

            loop_ctx = (
                tc.For_i(0, loop_reps, 1)
                if loop_reps > 1
                else contextlib.nullcontext()
            )
            with loop_ctx:
                main_gemm(
                    nc, tc, xqv, xcv, wh, wd, out, psA, psB, xpool, opool,
                    n_blocks, reps,
                )
    return nc


def main_gemm(nc, tc, xqv, xcv, wh, wd, out, psA, psB, xpool, opool, n_blocks, reps):
    for mb_r in range(n_blocks * reps):
        mb = mb_r % n_blocks
        xqb = xpool.tile([P, KT, MB], F8, tag="xqb")
        xcb = xpool.tile([P, KT, MB], F8, tag="xcb")
        for kc in range(4):  # split each 2MB block across DMA queues
            sl = slice(kc * 8, (kc + 1) * 8)
            msl = slice(mb * MB, (mb + 1) * MB)
            nc.sync.dma_start(xqb[:, sl, :], xqv[:, sl, msl])
            nc.sync.dma_start(xcb[:, sl, :], xcv[:, sl, msl])
        for j in range(MB // P):
            jsl = slice(j * P, (j + 1) * P)
            osb = opool.tile([P, NC], mybir.dt.float16, tag="osb")
            for c0, c1 in CHUNKS:
                cw = c1 - c0
                p1 = psA.tile([P, 512], mybir.dt.float32, name="p1")[:, :cw]
                p2 = psB.tile([P, 512], mybir.dt.float32, name="p2")[:, :cw]
                for t in range(KP):
                    ksl = slice(2 * t, 2 * t + 2)
                    nc.tensor.matmul(
                        p1[:],
                        lhsT=xqb[:, ksl, jsl],
                        rhs=wh[:, ksl, c0:c1],
                        start=(t == 0),
                        stop=(t == KP - 1),
                        perf_mode=mybir.MatmulPerfMode.DoubleRow,
                    )
                for t in range(KP):
                    ksl = slice(2 * t, 2 * t + 2)
                    nc.tensor.matmul(
                        p2[:],
                        lhsT=xcb[:, ksl, jsl],
                        rhs=wd[:, ksl, c0:c1],
                        start=(t == 0),
                        stop=(t == KP - 1),
                        perf_mode=mybir.MatmulPerfMode.DoubleRow,
                    )
                # osb = LAM*p1 + p2  (two DVE ops; replaces the copy the
                # fp16 version needed anyway)
                nc.vector.tensor_scalar(
                    out=osb[:, c0:c1],
                    in0=p1[:],
                    scalar1=LAM,
                    scalar2=None,
                    op0=mybir.AluOpType.mult,
                )
                nc.vector.tensor_tensor(
                    out=osb[:, c0:c1],
                    in0=osb[:, c0:c1],
                    in1=p2[:],
                    op=mybir.AluOpType.add,
                )
            m0 = mb * MB + j * P
            nc.sync.dma_start(out[m0 : m0 + P, :], osb[:])


def prep_inputs(x, weight_packed, scales):
    """Host-side shard/layout/fp8 prep. Returns per-core input maps."""
    import ml_dtypes

    E4 = ml_dtypes.float8_e4m3

    x = np.asarray(x)
    weight_packed = np.asarray(weight_packed)
    scales = np.asarray(scales, dtype=np.float16)

    m_total = x.shape[0] * x.shape[1]
    x2d = x.reshape(m_total, K).astype(np.float32)
    xh = x2d.astype(E4)
    xc = ((x2d - xh.astype(np.float32)) + BETA * xh.astype(np.float32)).astype(E4)
    xqT = np.ascontiguousarray(xh.T)  # (K, M) fp8
    xcT = np.ascontiguousarray(xc.T)

    wp8 = weight_packed.astype(np.uint8)  # (N, K//2), one byte per int32
    q = np.empty((N, K), dtype=np.float32)
    q[:, 0::2] = (wp8 & 0x0F).astype(np.float32)
    q[:, 1::2] = (wp8 >> 4).astype(np.float32)
    s_rep = np.repeat(scales.astype(np.float32), GROUP, axis=1)  # (N, K)
    w = (q - 8.0) * s_rep
    whq = w.astype(E4)
    whf = whq.astype(np.float32)
    # d = fp8(w_hi + w_lo/beta) = fp8(8w - 7w_hi) for beta = 1/8
    wd = (w / BETA - (1.0 / BETA - 1.0) * whf).astype(E4)

    in_maps = []
    for c in range(NCORES):
        nsl = slice(c * NC, (c + 1) * NC)
        in_maps.append(
            {
                "xqT": xqT,
                "xcT": xcT,
                "whT": np.ascontiguousarray(whq[nsl].T),  # (K, NC)
                "wdT": np.ascontiguousarray(wd[nsl].T),
            }
        )
    return in_maps


_program_cache: dict[int, bass.Bass] = {}


def get_program(m_total: int = M) -> bass.Bass:
    if m_total not in _program_cache:
        _program_cache[m_total] = build_program(m_total)
    return _program_cache[m_total]


def kernel(x, weight_packed, scales):
    import os

    os.environ.setdefault("NEURON_RT_RESET_CORES", "1")
    from concourse.bass_utils import run_bass_kernel_spmd

    x = np.asarray(x)
    in_maps = prep_inputs(x, weight_packed, scales)

    res = None
    last_exc = None
    for attempt in range(3):
        try:
            res = run_bass_kernel_spmd(get_program(), in_maps, list(range(NCORES)))
            break
        except Exception as e:  # transient NRT_EXEC_UNIT_UNRECOVERABLE flakes
            last_exc = e
            try:
                import jax

                jax.clear_caches()
            except Exception:
                pass
            import time

            time.sleep(10)
    if res is None:
        raise last_exc

    outs = [np.asarray(res.results[c]["out"]) for c in range(NCORES)]
    out2d = np.concatenate(outs, axis=1)  # (M, N) fp16
    return out2d.reshape(x.shape[0], x.shape[1], N)
